# revision 3
# baseline (speedup 1.0000x reference)
"""ExtractSearchWindows Trainium2 kernel (8 NeuronCores, Bass/Tile).

out[b, h, w, dy*cv+dx, ky*8+kx] = uint8(P[b, h+off+dy+ky, w+off+dx+kx])
with P = zero-pad(inputs[:, 0], 7) and off = 3 - search_range.

The output (196.6 MB u8) is a pure byte-replication of a tiny input, so
the kernel is bound by per-core DMA-engine write bandwidth (~425 GB/s
across 16 engines; ~26.6 GB/s/engine for descriptors >= 4 KB, less for
small ones).  Work is sharded over (b, h): each of the 8 cores produces
48 output rows as 384 segments (segment = 40-pixel row chunk) in 3
tiles of 128 partitions.

Device-side expansion: strided uint32 DVE tensor_copies read host-
prepared byte-shifted sub-rows S[seg][v][u][j] (v = dy+ky source row,
u = phi+dx byte shift, j = 4a+4kxp+beta addressing pixel w = 4a+phi,
kx = 4*kxp+beta) and scatter them into out-staging tiles that DMA out
with large contiguous descriptors.

Pipeline fill: a small fast-start slice S0a is DMA'd first so the DVE
starts ~1 us earlier; pixels 0-11 of tile 0 drain via two dy-sliced
blocks (640/960 B descriptors, ~0.73x engine rate -- paid while the
engines would otherwise idle); everything later uses w-chunks with
19.2-32 KB descriptors at full rate, sized so the engines never
starve once the first block lands.
"""
import numpy as np

K = 8
MAX_SR = 3
B, H, W = 2, 192, 320
TP = MAX_SR + K // 2          # 7 pad per side
PW = W + 2 * TP               # 334
NCORES = 8
ROWS_PER_CORE = (B * H) // NCORES   # 48
WSEG = 40
NWSEG = W // WSEG             # 8
NSEG = ROWS_PER_CORE * NWSEG  # 384
NTILE = NSEG // 128           # 3

# sr=2 geometry
CV = 5
OSEG = WSEG * CV * CV * K * K   # 64000 output bytes per segment
PIXB = CV * CV * K * K          # 1600 output bytes per pixel
PIXW = PIXB // 4                # 400 u32 per pixel
DW = CV * K * K // 4            # 80 u32 per (pixel, dy)

NV = 12                       # source rows per segment (CV-1+K)
NU = 8                        # byte shifts u = phi+dx
NJ = 44                       # shifted sub-row bytes
SEGB = NV * NU * NJ           # 4224 S bytes per segment
A_NV, A_NJ = 12, 16           # fast-start slice: all v, j<=15 (a<=1)
A_B = A_NV * NU * A_NJ        # 1536
RJ = 56                       # compact row bytes (covers u+j <= 50)
RB = NV * RJ                  # 672 compact bytes per segment

# persistent SBUF layout (u8 offsets)
S0A_OFF = 0
S_OFF = A_B                   # S tiles at S_OFF + t*SEGB
R12_OFF = S_OFF + NTILE * SEGB
PERS_B = R12_OFF + 2 * RB

import os
SPLIT_QUEUES = os.environ.get("ESW_SPLIT_QUEUES", "0") == "1"

_PROG_CACHE = {}


def _make_host_arrays(x, sr):
    """x: (B,1,H,W) f32 -> per-core dict of host-prepped u8 arrays."""
    off = MAX_SR - sr
    P = np.pad(x[:, 0], ((0, 0), (TP, TP), (TP, TP))).astype(np.uint8)
    cores = []
    st = np.lib.stride_tricks.as_strided
    for c in range(NCORES):
        b = (c * ROWS_PER_CORE) // H
        h0 = (c * ROWS_PER_CORE) % H
        flat = np.ascontiguousarray(P[b]).reshape(-1)
        base = (h0 + off) * PW + off
        # S: tile-0 segments fully shifted: (r, s, v, u, j)
        s = st(flat[base:], shape=(16, NWSEG, NV, NU, NJ),
               strides=(PW, WSEG, PW, 1, 1))
        s = np.ascontiguousarray(s).reshape(128, SEGB)
        # S0a: fast-start slice of tile 0 (all v, j<16)
        s0a = st(flat[base:], shape=(16, NWSEG, A_NV, NU, A_NJ),
                 strides=(PW, WSEG, PW, 1, 1))
        s0a = np.ascontiguousarray(s0a).reshape(128, A_B)
        # R12: compact un-shifted rows for tiles 1,2: (t, r, s, v, j)
        r12 = st(flat[base + 16 * PW:], shape=(2, 16, NWSEG, NV, RJ),
                 strides=(16 * PW, PW, WSEG, PW, 1))
        r12 = np.ascontiguousarray(r12.transpose(1, 2, 0, 3, 4)) \
            .reshape(128, 2 * RB)
        cores.append({"s0a": s0a, "s": s, "r12": r12})
    return cores


def _build_program(sr):
    import concourse.bass as bass
    import concourse.bacc as bacc
    import concourse.mybir as mybir
    from concourse import tile

    u8 = mybir.dt.uint8
    u16 = mybir.dt.uint16
    u32 = mybir.dt.uint32
    nc = bacc.Bacc("TRN2", debug=False)
    s0a_in = nc.declare_dram_parameter("s0a", [128, A_B], u8, isOutput=False)
    s_in = nc.declare_dram_parameter("s", [128, SEGB], u8, isOutput=False)
    r12_in = nc.declare_dram_parameter("r12", [128, 2 * RB], u8,
                                       isOutput=False)
    out = nc.declare_dram_parameter("out", [NSEG * OSEG], u8, isOutput=True)

    with tile.TileContext(nc) as tc:
        with tc.tile_pool(name="spool", bufs=1) as sp, \
             tc.tile_pool(name="tpool", bufs=1) as tp:
            PS = sp.tile([128, PERS_B], u8)
            p8 = PS[:]
            p16 = PS[:].bitcast(u16)
            p32 = PS[:].bitcast(u32)
            PP8, PP16, PP32 = PERS_B, PERS_B // 2, PERS_B // 4

            # host data in, latency-critical first, all on the SP queue
            nc.sync.dma_start(PS[:, S0A_OFF:S0A_OFF + A_B], s0a_in[:, :])
            nc.sync.dma_start(PS[:, R12_OFF:R12_OFF + 2 * RB], r12_in[:, :])
            nc.sync.dma_start(PS[:, S_OFF:S_OFF + SEGB], s_in[:, :])

            def build_s(t, parts):
                """Shift compact rows R into S[t]: S[t][v][u][j] =
                R[t][v][u+j].  Even u (u32/u16) on DVE, odd u (byte
                shifts) on the otherwise-idle Activation engine."""
                rb8 = R12_OFF + (t - 1) * RB
                s8 = S_OFF + t * SEGB
                for u in range(NU):
                    if u % 2 == 0 and parts == "even":
                        if u % 4 == 0:
                            src_ = bass.AP(p32.tensor, rb8 // 4 + u // 4,
                                           [[PP32, 128], [RJ // 4, NV],
                                            [1, NJ // 4]])
                            dst_ = bass.AP(p32.tensor,
                                           s8 // 4 + u * (NJ // 4),
                                           [[PP32, 128], [NU * NJ // 4, NV],
                                            [1, NJ // 4]])
                            nc.vector.tensor_copy(dst_, src_)
                        else:
                            src_ = bass.AP(p16.tensor, rb8 // 2 + u // 2,
                                           [[PP16, 128], [RJ // 2, NV],
                                            [1, NJ // 2]])
                            dst_ = bass.AP(p16.tensor,
                                           s8 // 2 + u * (NJ // 2),
                                           [[PP16, 128], [NU * NJ // 2, NV],
                                            [1, NJ // 2]])
                            nc.vector.tensor_copy(dst_, src_)
                    elif u % 2 == 1 and parts == "odd":
                        src_ = bass.AP(p8.tensor, rb8 + u,
                                       [[PP8, 128], [RJ, NV], [1, NJ]])
                        dst_ = bass.AP(p8.tensor, s8 + u * NJ,
                                       [[PP8, 128], [NU * NJ, NV], [1, NJ]])
                        nc.scalar.copy(dst_, src_)

            # odd-byte shifts start as soon as R12 lands
            build_s(1, "odd")
            build_s(2, "odd")

            def expand(s_off32, src_st, T, t_pitch32, pix_w32, dys, dy0,
                       a0, an):
                """DVE scatter block: one copy per (dy in dys, phi 0..3).

                Reads S at u32 offset s_off32 (+ dy*sv + phi*su + a*sa),
                writes staging tile T laid out [pixel][dy-dy0][dx][ky][kx]
                with pix_w32 u32 per pixel.
                """
                sv, su, sa = src_st
                t32 = T[:].bitcast(u32)
                for dy in dys:
                    for phi in range(4):
                        src = bass.AP(
                            p32.tensor,
                            s_off32 + dy * sv + phi * su + a0 * sa,
                            [[PP32, 128],
                             [sv, K],           # ky
                             [sa, an],          # a
                             [su, CV],          # dx
                             [1, 2]])           # kx pair
                        dst = bass.AP(
                            t32.tensor,
                            phi * pix_w32 + (dy - dy0) * DW,
                            [[t_pitch32, 128],
                             [2, K],                    # ky
                             [4 * pix_w32, an],         # a
                             [K * K // 4, CV],          # dx
                             [1, 2]])                   # kx pair
                        nc.vector.tensor_copy(dst, src)

            A_ST = (NU * A_NJ // 4, A_NJ // 4, 1)
            S_ST = (NU * NJ // 4, NJ // 4, 1)

            def s_off32(t):
                return (S_OFF + t * SEGB) // 4

            def wchunk(t, a0, an, bufs, tag, split=False):
                """Full-depth w-chunk: pixels 4*a0 .. 4*(a0+an)-1 of tile t."""
                T = tp.tile([128, 20 * PIXB], u8, bufs=bufs, name=tag)
                expand(s_off32(t), S_ST, T, 20 * PIXW, PIXW,
                       (0, 1, 2, 3, 4), 0, a0, an)
                nb = 4 * an * PIXB
                if not split:
                    nc.sync.dma_start(
                        bass.AP(out.ap().tensor,
                                t * 128 * OSEG + 4 * a0 * PIXB,
                                [[OSEG, 128], [1, nb]]),
                        T[0:128, 0:nb])
                else:
                    h = nb // 2
                    for i, eng in enumerate((nc.sync, nc.scalar)):
                        eng.dma_start(
                            bass.AP(out.ap().tensor,
                                    t * 128 * OSEG + 4 * a0 * PIXB + i * h,
                                    [[OSEG, 128], [1, h]]),
                            T[0:128, i * h:(i + 1) * h])

            # ---- tile 0 fill --------------------------------------------
            # g1: dy{0,1} x px 0-11 from the fast-start slice (640 B descs)
            Tg1 = tp.tile([128, 12 * 640], u8, bufs=1)
            expand(S0A_OFF // 4, A_ST, Tg1, 12 * 160, 2 * DW, (0, 1), 0,
                   0, 3)
            nc.sync.dma_start(
                bass.AP(out.ap().tensor, 0,
                        [[OSEG, 128], [PIXB, 12], [1, 640]]),
                Tg1[0:128, 0:7680])
            # g2: dy{2,3,4} x px 0-11, also from the slice (960 B descs)
            Tg2 = tp.tile([128, 12 * 960], u8, bufs=1)
            expand(S0A_OFF // 4, A_ST, Tg2, 12 * 240, 3 * DW, (2, 3, 4), 2,
                   0, 3)
            nc.sync.dma_start(
                bass.AP(out.ap().tensor, 640,
                        [[OSEG, 128], [PIXB, 12], [1, 960]]),
                Tg2[0:128, 0:11520])
            # g3/g4: px 12-27, 28-39 full-depth w-chunks
            wchunk(0, 3, 4, 5, "Tst", split=SPLIT_QUEUES)
            wchunk(0, 7, 3, 5, "Tst", split=SPLIT_QUEUES)

            # ---- steady tiles 1,2: 20px w-chunks ------------------------
            for t in (1, 2):
                build_s(t, "even")
                for ch in range(2):
                    wchunk(t, 5 * ch, 5, 5, "Tst", split=SPLIT_QUEUES)
    nc.compile()
    return nc


def _numpy_fallback(x, sr):
    cv = 2 * sr + 1
    off = MAX_SR - sr
    P = np.pad(x[:, 0], ((0, 0), (TP, TP), (TP, TP))).astype(np.uint8)
    out = np.empty((B, H, W, cv * cv, K * K), np.uint8)
    for dy in range(cv):
        for dx in range(cv):
            for ky in range(K):
                for kx in range(K):
                    out[:, :, :, dy * cv + dx, ky * K + kx] = \
                        P[:, off + dy + ky:off + dy + ky + H,
                          off + dx + kx:off + dx + kx + W]
    return out


def kernel(inputs, search_range):
    from concourse.bass_utils import run_bass_kernel_spmd

    x = np.asarray(inputs, dtype=np.float32)
    sr = int(np.asarray(search_range))
    if sr != 2 or x.shape != (B, 1, H, W):
        return _numpy_fallback(x, sr)

    if sr not in _PROG_CACHE:
        _PROG_CACHE[sr] = _build_program(sr)
    nc = _PROG_CACHE[sr]

    host = _make_host_arrays(x, sr)
    res = run_bass_kernel_spmd(nc, host, list(range(NCORES)))
    outs = [np.asarray(res.results[c]["out"]) for c in range(NCORES)]
    return np.concatenate(outs).reshape(B, H, W, CV * CV, K * K)



# revision 4
# speedup vs baseline: 1.0319x; 1.0319x over previous
"""ExtractSearchWindows Trainium2 kernel (8 NeuronCores, Bass/Tile).

out[b, h, w, dy*cv+dx, ky*8+kx] = uint8(P[b, h+off+dy+ky, w+off+dx+kx])
with P = zero-pad(inputs[:, 0], 7) and off = 3 - search_range.

The output (196.6 MB u8) is a pure byte-replication of a tiny input, so
the kernel is bound by SBUF-AXI / SDMA write bandwidth (16 engines x
~27.2 GB/s = ~435 GB/s per core).  Work is sharded over (b, h): each of
the 8 cores produces 48 output rows as 384 segments (segment = 40-pixel
row chunk) in 3 tiles of 128 partitions.

Expansion uses dy-fused per-pixel DVE copies: one 4-dim tensor_copy per
output pixel covers all (dy, ky, dx, kx) at once by exploiting that dy
and ky address source rows with the SAME stride (overlapping reads),
400 u32 per partition per copy.  DVE issue rate ~50ns + 0.62ns/elem
=> ~690 B/s/ns aggregate, comfortably above the ~430 B/ns DMA drain.

Ramp: a widened fast-start slice S0A (j<24, covering pixels 0-19 of
tile 0) is DMA'd first; output pieces grow geometrically (2,3,5,8,12,10
pixels) so the SDMA engines never idle after the input loads drain; the
full tile-0 S loads on the scalar HWDGE ring in two 2112B-descriptor
halves so it streams concurrently with the first output pieces.  Tiles
1-2 use host-compacted rows (R12) shifted on-device (odd byte shifts on
the Activation engine, even shifts as u32/u16 DVE copies).  All output
descriptors are >= 3200 B (>= 25 B/ns/engine vs 19-20.5 for the
sub-1KB descriptors the previous version used during the ramp).
"""
import numpy as np

K = 8
MAX_SR = 3
B, H, W = 2, 192, 320
TP = MAX_SR + K // 2          # 7 pad per side
PW = W + 2 * TP               # 334
NCORES = 8
ROWS_PER_CORE = (B * H) // NCORES   # 48
WSEG = 40
NWSEG = W // WSEG             # 8
NSEG = ROWS_PER_CORE * NWSEG  # 384
NTILE = NSEG // 128           # 3

# sr=2 geometry
CV = 5
OSEG = WSEG * CV * CV * K * K   # 64000 output bytes per segment
PIXB = CV * CV * K * K          # 1600 output bytes per pixel
PIXW = PIXB // 4                # 400 u32 per pixel
DW = CV * K * K // 4            # 80 u32 per (pixel, dy)

NV = 12                       # source rows per segment (CV-1+K)
NU = 8                        # byte shifts u = phi+dx
NJ = 44                       # shifted sub-row bytes
SEGB = NV * NU * NJ           # 4224 S bytes per segment
A_NV, A_NJ = 12, 24           # fast-start slice: all v, j<24 (a<=4 ok)
A_B = A_NV * NU * A_NJ        # 2304
RJ = 56                       # compact row bytes (covers u+j <= 50)
RB = NV * RJ                  # 672 compact bytes per segment

# output piece sizes (pixels) per tile: geometric ramp for tile 0,
# full 20-px w-chunks for tiles 1-2.
PIECES_T0 = (2, 3, 5, 8, 12, 10)
PIECES_T12 = (20, 20)

_PROG_CACHE = {}


def _make_host_arrays(x, sr):
    """x: (B,1,H,W) f32 -> per-core dict of host-prepped u8 arrays."""
    off = MAX_SR - sr
    P = np.pad(x[:, 0], ((0, 0), (TP, TP), (TP, TP))).astype(np.uint8)
    cores = []
    st = np.lib.stride_tricks.as_strided
    for c in range(NCORES):
        b = (c * ROWS_PER_CORE) // H
        h0 = (c * ROWS_PER_CORE) % H
        flat = np.ascontiguousarray(P[b]).reshape(-1)
        base = (h0 + off) * PW + off
        # S: tile-0 segments fully shifted: (r, s, v, u, j)
        s = st(flat[base:], shape=(16, NWSEG, NV, NU, NJ),
               strides=(PW, WSEG, PW, 1, 1))
        s = np.ascontiguousarray(s).reshape(128, SEGB)
        # S0a: fast-start slice of tile 0 (all v, j<24 -> pixels 0-19)
        s0a = st(flat[base:], shape=(16, NWSEG, A_NV, NU, A_NJ),
                 strides=(PW, WSEG, PW, 1, 1))
        s0a = np.ascontiguousarray(s0a).reshape(128, A_B)
        # R12: compact un-shifted rows for tiles 1,2: (t, r, s, v, j)
        r12 = st(flat[base + 16 * PW:], shape=(2, 16, NWSEG, NV, RJ),
                 strides=(16 * PW, PW, WSEG, PW, 1))
        r12 = np.ascontiguousarray(r12.transpose(1, 2, 0, 3, 4)) \
            .reshape(128, 2 * RB)
        cores.append({"s0a": s0a, "s": s, "r12": r12})
    return cores


def _build_program(sr):
    import concourse.bass as bass
    import concourse.bacc as bacc
    import concourse.mybir as mybir
    from concourse import tile

    u8 = mybir.dt.uint8
    u16 = mybir.dt.uint16
    u32 = mybir.dt.uint32
    nc = bacc.Bacc("TRN2", debug=False)
    s0a_in = nc.declare_dram_parameter("s0a", [128, A_B], u8, isOutput=False)
    s_in = nc.declare_dram_parameter("s", [128, SEGB], u8, isOutput=False)
    r12_in = nc.declare_dram_parameter("r12", [128, 2 * RB], u8,
                                       isOutput=False)
    out = nc.declare_dram_parameter("out", [NSEG * OSEG], u8, isOutput=True)

    with tile.TileContext(nc) as tc:
        with tc.tile_pool(name="spool", bufs=1) as sp, \
             tc.tile_pool(name="tpool", bufs=1) as tp:
            S0A = sp.tile([128, A_B], u8)
            S0 = sp.tile([128, SEGB], u8)
            S1 = sp.tile([128, SEGB], u8)
            S2 = sp.tile([128, SEGB], u8)
            R12 = sp.tile([128, 2 * RB], u8)

            # input DMAs: fast-start slice + compact rows on the SP ring;
            # tile-0 full S on the scalar HWDGE ring (2112B descriptors)
            # so it streams while the first output pieces drain.
            nc.sync.dma_start(S0A[:, :], s0a_in[:, :])
            nc.sync.dma_start(R12[:, :], r12_in[:, :])
            half = SEGB // 2
            nc.scalar.dma_start(S0[:, 0:half], s_in[:, 0:half])
            nc.scalar.dma_start(S0[:, half:SEGB], s_in[:, half:SEGB])

            S_tiles = (S0, S1, S2)

            def build_s_odd(t):
                """Odd byte shifts R12 -> S[t] on the Activation engine."""
                s8 = S_tiles[t][:]
                r8 = R12[:]
                for u in (1, 3, 5, 7):
                    src = bass.AP(r8.tensor, (t - 1) * RB + u,
                                  [[2 * RB, 128], [RJ, NV], [1, NJ]])
                    dst = bass.AP(s8.tensor, u * NJ,
                                  [[SEGB, 128], [NU * NJ, NV], [1, NJ]])
                    nc.scalar.copy(dst, src)

            def build_s_even(t):
                """Even shifts R12 -> S[t]: u32 (u=0,4) + u16 (u=2,6) DVE."""
                s32 = S_tiles[t][:].bitcast(u32)
                s16 = S_tiles[t][:].bitcast(u16)
                r32 = R12[:].bitcast(u32)
                r16 = R12[:].bitcast(u16)
                for u in (0, 4):
                    src = bass.AP(r32.tensor, (t - 1) * (RB // 4) + u // 4,
                                  [[2 * RB // 4, 128], [RJ // 4, NV],
                                   [1, NJ // 4]])
                    dst = bass.AP(s32.tensor, u * (NJ // 4),
                                  [[SEGB // 4, 128], [NU * NJ // 4, NV],
                                   [1, NJ // 4]])
                    nc.vector.tensor_copy(dst, src)
                for u in (2, 6):
                    src = bass.AP(r16.tensor, (t - 1) * (RB // 2) + u // 2,
                                  [[2 * RB // 2, 128], [RJ // 2, NV],
                                   [1, NJ // 2]])
                    dst = bass.AP(s16.tensor, u * (NJ // 2),
                                  [[SEGB // 2, 128], [NU * NJ // 2, NV],
                                   [1, NJ // 2]])
                    nc.vector.tensor_copy(dst, src)

            def copy_px(T, slot, t, px):
                """One dy-fused copy: all (dy,ky,dx,kx) of pixel px of
                tile t into T at pixel-slot `slot` (400 u32/partition)."""
                a, phi = px // 4, px % 4
                if t == 0 and px < 20:
                    stile, su, pp = S0A, A_NJ // 4, A_B // 4
                else:
                    stile, su, pp = S_tiles[t], NJ // 4, SEGB // 4
                sv = NU * su
                s32 = stile[:].bitcast(u32)
                t32 = T[:].bitcast(u32)
                src = bass.AP(s32.tensor, phi * su + a,
                              [[pp, 128], [sv, CV], [sv, K], [su, CV],
                               [1, 2]])
                dst = bass.AP(t32.tensor, slot * PIXW,
                              [[20 * PIXW, 128], [DW, CV], [2, K],
                               [K * K // 4, CV], [1, 2]])
                nc.vector.tensor_copy(dst, src)

            # build_s odd shifts start as soon as R12 lands (ACT order:
            # after its two s-half dma triggers).
            build_s_odd(1)
            build_s_odd(2)

            def piece(t, p0, n):
                T = tp.tile([128, 20 * PIXB], u8, bufs=4, name="Tst")
                for i in range(n):
                    copy_px(T, i, t, p0 + i)
                nc.sync.dma_start(
                    bass.AP(out.ap().tensor, t * 128 * OSEG + p0 * PIXB,
                            [[OSEG, 128], [1, n * PIXB]]),
                    T[0:128, 0:n * PIXB])

            p0 = 0
            for n in PIECES_T0:
                piece(0, p0, n)
                p0 += n
            for t in (1, 2):
                build_s_even(t)
                p0 = 0
                for n in PIECES_T12:
                    piece(t, p0, n)
                    p0 += n
    nc.compile()
    return nc


def _numpy_fallback(x, sr):
    cv = 2 * sr + 1
    off = MAX_SR - sr
    P = np.pad(x[:, 0], ((0, 0), (TP, TP), (TP, TP))).astype(np.uint8)
    out = np.empty((B, H, W, cv * cv, K * K), np.uint8)
    for dy in range(cv):
        for dx in range(cv):
            for ky in range(K):
                for kx in range(K):
                    out[:, :, :, dy * cv + dx, ky * K + kx] = \
                        P[:, off + dy + ky:off + dy + ky + H,
                          off + dx + kx:off + dx + kx + W]
    return out


def kernel(inputs, search_range):
    from concourse.bass_utils import run_bass_kernel_spmd

    x = np.asarray(inputs, dtype=np.float32)
    sr = int(np.asarray(search_range))
    if sr != 2 or x.shape != (B, 1, H, W):
        return _numpy_fallback(x, sr)

    if sr not in _PROG_CACHE:
        _PROG_CACHE[sr] = _build_program(sr)
    nc = _PROG_CACHE[sr]

    host = _make_host_arrays(x, sr)
    res = run_bass_kernel_spmd(nc, host, list(range(NCORES)))
    outs = [np.asarray(res.results[c]["out"]) for c in range(NCORES)]
    return np.concatenate(outs).reshape(B, H, W, CV * CV, K * K)


# revision 5
# speedup vs baseline: 1.2034x; 1.1662x over previous
"""ExtractSearchWindows Trainium2 kernel (8 NeuronCores, Bass/Tile).

out[b, h, w, dy*cv+dx, ky*8+kx] = uint8(P[b, h+off+dy+ky, w+off+dx+kx])
with P = zero-pad(inputs[:, 0], 7) and off = 3 - search_range.

The output (196.6 MB u8) is a pure byte-replication of a tiny input, so
the kernel is bound by SBUF-AXI / SDMA write bandwidth (16 engines x
~27.2 GB/s = ~435 GB/s per core).  Work is sharded over (b, h): each of
the 8 cores produces 48 output rows as 384 segments (segment = 40-pixel
row chunk) in 3 tiles of 128 partitions.

Expansion uses dy-fused per-pixel DVE copies: one 4-dim tensor_copy per
output pixel covers all (dy, ky, dx, kx) at once by exploiting that dy
and ky address source rows with the SAME stride (overlapping reads),
400 u32 per partition per copy at ~300-330 ns issue spacing (~640 B/ns
aggregate), above the ~430 B/ns DMA drain rate.

Ramp: pixels 0-3 of tile 0 are fully host-expanded (tpre) and written
straight DRAM->DRAM as the second DMA -- engine-time-neutral vs the
SBUF path but available immediately, so the SDMA engines never idle
while the first DVE pieces are prepared.  A fast-start slice S0A
(j<24: pixels 0-19) and a tail slice S_HI (j in [20,44): pixels 20-39)
feed tile 0; tiles 1-2 use host-compacted rows (R12) shifted on-device
(odd byte shifts on the Activation engine, even as u32/u16 DVE copies).
Output piece sizes grow geometrically (2,3,4,6,9,12 px after the 4
pre-written) so engines stream gap-free; descriptors are all >= 3200 B
(>= 25 B/ns/engine).
"""
import numpy as np

K = 8
MAX_SR = 3
B, H, W = 2, 192, 320
TP = MAX_SR + K // 2          # 7 pad per side
PW = W + 2 * TP               # 334
NCORES = 8
ROWS_PER_CORE = (B * H) // NCORES   # 48
WSEG = 40
NWSEG = W // WSEG             # 8
NSEG = ROWS_PER_CORE * NWSEG  # 384
NTILE = NSEG // 128           # 3

# sr=2 geometry
CV = 5
OSEG = WSEG * CV * CV * K * K   # 64000 output bytes per segment
PIXB = CV * CV * K * K          # 1600 output bytes per pixel
PIXW = PIXB // 4                # 400 u32 per pixel
DW = CV * K * K // 4            # 80 u32 per (pixel, dy)

NV = 12                       # source rows per segment (CV-1+K)
NU = 8                        # byte shifts u = phi+dx
NJ = 44                       # shifted sub-row bytes
SEGB = NV * NU * NJ           # 4224 S bytes per segment
A_NJ = 24                     # fast-start slice: j<24 (covers px 0-19)
A_B = NV * NU * A_NJ          # 2304
H_J0 = 20                     # tail slice: j in [20, 44) (covers px 20-39)
H_NJ = NJ - H_J0              # 24
H_B = NV * NU * H_NJ          # 2304
RJ = 56                       # compact row bytes (covers u+j <= 50)
RB = NV * RJ                  # 672 compact bytes per segment

NPRE = 4                      # pixels of tile 0 pre-expanded on host
PREB = NPRE * PIXB            # 6400 bytes per partition

# output piece sizes (pixels) per tile: geometric ramp for tile 0
# (after the NPRE pre-written pixels), full 20-px chunks for tiles 1-2.
PIECES_T0 = (2, 3, 4, 6, 9, 12)
PIECES_T12 = (20, 20)

_PROG_CACHE = {}


def _make_host_arrays(x, sr):
    """x: (B,1,H,W) f32 -> per-core dict of host-prepped u8 arrays."""
    off = MAX_SR - sr
    P = np.pad(x[:, 0], ((0, 0), (TP, TP), (TP, TP))).astype(np.uint8)
    cores = []
    st = np.lib.stride_tricks.as_strided
    for c in range(NCORES):
        b = (c * ROWS_PER_CORE) // H
        h0 = (c * ROWS_PER_CORE) % H
        flat = np.ascontiguousarray(P[b]).reshape(-1)
        base = (h0 + off) * PW + off
        # S0a: fast-start slice of tile 0 (all v, j<24 -> pixels 0-19)
        s0a = st(flat[base:], shape=(16, NWSEG, NV, NU, A_NJ),
                 strides=(PW, WSEG, PW, 1, 1))
        s0a = np.ascontiguousarray(s0a).reshape(128, A_B)
        # S_hi: tail slice of tile 0 (all v, j in [20,44) -> pixels 20-39)
        s_hi = st(flat[base + H_J0:], shape=(16, NWSEG, NV, NU, H_NJ),
                  strides=(PW, WSEG, PW, 1, 1))
        s_hi = np.ascontiguousarray(s_hi).reshape(128, H_B)
        # R12: compact un-shifted rows for tiles 1,2: (t, r, s, v, j)
        r12 = st(flat[base + 16 * PW:], shape=(2, 16, NWSEG, NV, RJ),
                 strides=(16 * PW, PW, WSEG, PW, 1))
        r12 = np.ascontiguousarray(r12.transpose(1, 2, 0, 3, 4)) \
            .reshape(128, 2 * RB)
        # tpre: host-expanded output bytes for pixels 0..NPRE-1 of tile 0:
        # (r, s, px, dy, dx, ky, kx) with px/dx/kx stride 1, dy/ky stride PW
        tpre = st(flat[base:], shape=(16, NWSEG, NPRE, CV, CV, K, K),
                  strides=(PW, WSEG, 1, PW, 1, PW, 1))
        tpre = np.ascontiguousarray(tpre).reshape(128, PREB)
        cores.append({"s0a": s0a, "s_hi": s_hi, "r12": r12, "tpre": tpre})
    return cores


def _build_program(sr):
    import concourse.bass as bass
    import concourse.bacc as bacc
    import concourse.mybir as mybir
    from concourse import tile

    u8 = mybir.dt.uint8
    u16 = mybir.dt.uint16
    u32 = mybir.dt.uint32
    nc = bacc.Bacc("TRN2", debug=False)
    s0a_in = nc.declare_dram_parameter("s0a", [128, A_B], u8, isOutput=False)
    s_hi_in = nc.declare_dram_parameter("s_hi", [128, H_B], u8,
                                        isOutput=False)
    r12_in = nc.declare_dram_parameter("r12", [128, 2 * RB], u8,
                                       isOutput=False)
    tpre_in = nc.declare_dram_parameter("tpre", [128, PREB], u8,
                                        isOutput=False)
    out = nc.declare_dram_parameter("out", [NSEG * OSEG], u8, isOutput=True)

    with tile.TileContext(nc) as tc:
        with tc.tile_pool(name="spool", bufs=1) as sp, \
             tc.tile_pool(name="tpool", bufs=1) as tp:
            S0A = sp.tile([128, A_B], u8)
            SHI = sp.tile([128, H_B], u8)
            S1 = sp.tile([128, SEGB], u8)
            S2 = sp.tile([128, SEGB], u8)
            R12 = sp.tile([128, 2 * RB], u8)

            # Input + pre-expanded DMAs.  SP ring order: s0a first (the
            # DVE dependency), then the DRAM->DRAM pre-written pixels
            # 0-3 of tile 0 (engines stay busy while DVE ramps), then
            # r12.  S_hi rides the scalar HWDGE ring in parallel.
            nc.sync.dma_start(S0A[:, :], s0a_in[:, :])
            nc.sync.dma_start(
                bass.AP(out.ap().tensor, 0, [[OSEG, 128], [1, PREB]]),
                tpre_in[:, :])
            nc.sync.dma_start(R12[:, :], r12_in[:, :])
            nc.scalar.dma_start(SHI[:, :], s_hi_in[:, :])

            S_tiles = (None, S1, S2)

            def build_s_odd(t):
                """Odd byte shifts R12 -> S[t] on the Activation engine."""
                s8 = S_tiles[t][:]
                r8 = R12[:]
                for u in (1, 3, 5, 7):
                    src = bass.AP(r8.tensor, (t - 1) * RB + u,
                                  [[2 * RB, 128], [RJ, NV], [1, NJ]])
                    dst = bass.AP(s8.tensor, u * NJ,
                                  [[SEGB, 128], [NU * NJ, NV], [1, NJ]])
                    nc.scalar.copy(dst, src)

            def build_s_even(t):
                """Even shifts R12 -> S[t]: u32 (u=0,4) + u16 (u=2,6) DVE."""
                s32 = S_tiles[t][:].bitcast(u32)
                s16 = S_tiles[t][:].bitcast(u16)
                r32 = R12[:].bitcast(u32)
                r16 = R12[:].bitcast(u16)
                for u in (0, 4):
                    src = bass.AP(r32.tensor, (t - 1) * (RB // 4) + u // 4,
                                  [[2 * RB // 4, 128], [RJ // 4, NV],
                                   [1, NJ // 4]])
                    dst = bass.AP(s32.tensor, u * (NJ // 4),
                                  [[SEGB // 4, 128], [NU * NJ // 4, NV],
                                   [1, NJ // 4]])
                    nc.vector.tensor_copy(dst, src)
                for u in (2, 6):
                    src = bass.AP(r16.tensor, (t - 1) * (RB // 2) + u // 2,
                                  [[2 * RB // 2, 128], [RJ // 2, NV],
                                   [1, NJ // 2]])
                    dst = bass.AP(s16.tensor, u * (NJ // 2),
                                  [[SEGB // 2, 128], [NU * NJ // 2, NV],
                                   [1, NJ // 2]])
                    nc.vector.tensor_copy(dst, src)

            def copy_px(T, slot, t, px):
                """One dy-fused copy: all (dy,ky,dx,kx) of pixel px of
                tile t into T at pixel-slot `slot` (400 u32/partition)."""
                a, phi = px // 4, px % 4
                if t == 0 and px < 20:
                    stile, su, pp = S0A, A_NJ // 4, A_B // 4
                    off32 = phi * su + a
                elif t == 0:
                    stile, su, pp = SHI, H_NJ // 4, H_B // 4
                    off32 = phi * su + (a - H_J0 // 4)
                else:
                    stile, su, pp = S_tiles[t], NJ // 4, SEGB // 4
                    off32 = phi * su + a
                sv = NU * su
                s32 = stile[:].bitcast(u32)
                t32 = T[:].bitcast(u32)
                src = bass.AP(s32.tensor, off32,
                              [[pp, 128], [sv, CV], [sv, K], [su, CV],
                               [1, 2]])
                dst = bass.AP(t32.tensor, slot * PIXW,
                              [[20 * PIXW, 128], [DW, CV], [2, K],
                               [K * K // 4, CV], [1, 2]])
                nc.vector.tensor_copy(dst, src)

            # build_s odd shifts start as soon as R12 lands (ACT order:
            # after its s_hi dma trigger).
            build_s_odd(1)
            build_s_odd(2)

            def piece(t, p0, n):
                T = tp.tile([128, 20 * PIXB], u8, bufs=4, name="Tst")
                for i in range(n):
                    copy_px(T, i, t, p0 + i)
                nc.sync.dma_start(
                    bass.AP(out.ap().tensor, t * 128 * OSEG + p0 * PIXB,
                            [[OSEG, 128], [1, n * PIXB]]),
                    T[0:128, 0:n * PIXB])

            p0 = NPRE
            for n in PIECES_T0:
                piece(0, p0, n)
                p0 += n
            for t in (1, 2):
                build_s_even(t)
                p0 = 0
                for n in PIECES_T12:
                    piece(t, p0, n)
                    p0 += n
    nc.compile()
    return nc


def _numpy_fallback(x, sr):
    cv = 2 * sr + 1
    off = MAX_SR - sr
    P = np.pad(x[:, 0], ((0, 0), (TP, TP), (TP, TP))).astype(np.uint8)
    out = np.empty((B, H, W, cv * cv, K * K), np.uint8)
    for dy in range(cv):
        for dx in range(cv):
            for ky in range(K):
                for kx in range(K):
                    out[:, :, :, dy * cv + dx, ky * K + kx] = \
                        P[:, off + dy + ky:off + dy + ky + H,
                          off + dx + kx:off + dx + kx + W]
    return out


def kernel(inputs, search_range):
    from concourse.bass_utils import run_bass_kernel_spmd

    x = np.asarray(inputs, dtype=np.float32)
    sr = int(np.asarray(search_range))
    if sr != 2 or x.shape != (B, 1, H, W):
        return _numpy_fallback(x, sr)

    if sr not in _PROG_CACHE:
        _PROG_CACHE[sr] = _build_program(sr)
    nc = _PROG_CACHE[sr]

    host = _make_host_arrays(x, sr)
    res = run_bass_kernel_spmd(nc, host, list(range(NCORES)))
    outs = [np.asarray(res.results[c]["out"]) for c in range(NCORES)]
    return np.concatenate(outs).reshape(B, H, W, CV * CV, K * K)
